# revision 38
# baseline (speedup 1.0000x reference)
"""Trainium2 Bass kernel for a single causal attention head.

Reference computation (per batch element b):
    Q = x_b @ wQ.T ; K = x_b @ wK.T ; V = x_b @ wV.T          [S, DK]
    P = softmax(causal_mask(Q @ K.T * d_model**-0.5))          [S, S]
    O = P @ V                                                  [S, DK]

Sharding: one batch element per NeuronCore (B == n_cores == 8).
Weights are replicated. No collectives needed.

Per-core device layout (host pre-transposes/casts for PE-friendly fp16):
    xt    [D, S]   fp16  x_b transposed (contraction dim D on partitions)
    wqk   [D, 128] fp16  [wQ.T | wK.T]  -> packed projection, M=128
    wv    [D, 64]  fp16  wV.T
    dmask [2, 128, 2*QC] fp16 causal masks for diagonal block-pairs
Output:
    o     [S, DK] fp32

Device pipeline per 512-wide q-chunk c (fp16 matmuls, fp32 PSUM):
  - DMA xt chunk; project Q^T,K^T packed (M=128); project V^T with
    col-tiled half-chunks (two concurrent M=64 matmuls).
  - Duplicate Q^T/K^T across both partition halves (SBUF->SBUF DMA) so
    score matmuls for t-tiles j,j+1 run row-packed (rows 0-63 / 64-127
    of the PE array concurrently), writing one [128,1024] PSUM pair.
  - One exp ACTIVATE per pair -> P~ (fp16, masked on diagonal pairs).
  - PV: accumulate [V_j | 1].T @ P~_j into PSUM (U^T rows 0-63,
    softmax denominators in row 64).
  - Epilogue: PE-transpose U^T, multiply by 1/rowsum, DMA out.
"""

import numpy as np
import ml_dtypes

B, S, D, DK = 8, 4096, 1024, 64
P = 128
QC = 512          # q-chunk width (matmul moving dim)
NQ = S // QC      # 8 q-chunks
ND = D // P       # 8 contraction chunks
NT = S // P       # 32 t-tiles
SCALE = float(D) ** -0.5   # 1/32
VW = 66           # per-t-tile stride in v_sb (64 V cols + 1 ones + pad)

# optimization knobs (validated on HW; flip off if a variant fails)
ROWPACK_SCORES = True   # row-packed score matmul pairs
COLPACK_V = True        # col-tiled V projection halves

_CACHED = {}


def _build_nc():
    import concourse.mybir as mybir
    import concourse.tile as tile
    from concourse import bacc
    from concourse.masks import make_identity
    from contextlib import ExitStack

    f32 = mybir.dt.float32
    f16 = mybir.dt.float16
    Exp = mybir.ActivationFunctionType.Exp
    mult = mybir.AluOpType.mult

    nc = bacc.Bacc()
    xt_h = nc.declare_dram_parameter("xt", [D, S], f16, isOutput=False)
    wqk_h = nc.declare_dram_parameter("wqk", [D, P], f16, isOutput=False)
    wkq_h = nc.declare_dram_parameter("wkq", [D, P], f16, isOutput=False)
    wv_h = nc.declare_dram_parameter("wv", [D, DK], f16, isOutput=False)
    ch_h = nc.declare_dram_parameter("chain", [P, P], f32, isOutput=False)
    o_h = nc.declare_dram_parameter("o", [S, DK], f32, isOutput=True)
    cho_h = nc.declare_dram_parameter("chain_o", [P, P], f32, isOutput=True)

    with tile.TileContext(nc) as tc, ExitStack() as ctx:
        const = ctx.enter_context(tc.tile_pool(name="const", bufs=1))
        xt_pool = ctx.enter_context(tc.tile_pool(name="xtp", bufs=3))
        pers = ctx.enter_context(tc.tile_pool(name="pers", bufs=1))
        pt_pool = ctx.enter_context(tc.tile_pool(name="ptp", bufs=4))
        stage = ctx.enter_context(tc.tile_pool(name="stage", bufs=3))
        # PSUM budget (8 banks): pair pool 2x2 + po 2 + sm 2x1 = 8
        ps_pair = ctx.enter_context(tc.tile_pool(name="ps_pair", bufs=2, space="PSUM"))
        ps_op = ctx.enter_context(tc.tile_pool(name="ps_op", bufs=2, space="PSUM"))
        ps_sm = ctx.enter_context(tc.tile_pool(name="ps_sm", bufs=2, space="PSUM"))

        # ---- x chunk 0 first (critical path), split so the projection can
        # start on the first half while the second streams in ----
        xtc0 = xt_pool.tile([P, ND, QC], f16, name="xtc", tag="xtc")
        for h in range(2):
            nc.sync.dma_start(
                out=xtc0[:, 4 * h:4 * (h + 1), :],
                in_=xt_h[4 * h * P:4 * (h + 1) * P, 0:QC].rearrange(
                    "(dc p) s -> p dc s", p=P
                ),
            )
        # ---- constants ----
        wqk_sb = const.tile([P, ND, P], f16)
        nc.sync.dma_start(out=wqk_sb, in_=wqk_h[:].rearrange("(dc p) m -> p dc m", p=P))
        wkq_sb = const.tile([P, ND, P], f16)
        nc.sync.dma_start(out=wkq_sb, in_=wkq_h[:].rearrange("(dc p) m -> p dc m", p=P))
        wv_sb = const.tile([P, ND, DK], f16)
        nc.sync.dma_start(out=wv_sb, in_=wv_h[:].rearrange("(dc p) m -> p dc m", p=P))
        # causal masks built on-device (a DMA here polluted the startup
        # DMA-sem lane and delayed the first projection)
        dm_sb = const.tile([P, 2, 2 * QC], f16)
        nc.gpsimd.memset(dm_sb, 0.0)
        for g in range(2):
            for h in range(2):
                nc.gpsimd.affine_select(
                    out=dm_sb[:, g, h * QC:(h + 1) * QC],
                    in_=dm_sb[:, g, h * QC:(h + 1) * QC],
                    compare_op=mybir.AluOpType.is_gt,
                    fill=1.0,
                    base=P * (2 * g + h),
                    pattern=[[-1, QC]],
                    channel_multiplier=1,
                )
        ident16 = const.tile([P, P], f16)
        make_identity(nc, ident16)
        ident32 = const.tile([P, P], f32)
        make_identity(nc, ident32)
        # tiny pass-through used to chain executions when benchmarking
        cht = const.tile([P, P], f32, name="cht")
        nc.sync.dma_start(out=cht, in_=ch_h[:])
        nc.sync.dma_start(out=cho_h[:], in_=cht)
        # ---- PE warm-up: ~3.5us of dummy matmuls during the initial DMA
        # wait so the HAM clock gate is already at full rate (2.4 GHz) when
        # the first projection runs ----
        warm_sb = const.tile([P, QC], f16, name="warm_sb")
        nc.vector.memset(warm_sb, 0.0)
        ps_warm = ps_sm.tile([P, QC], f32, name="ps_warm", tag="sm")
        for _ in range(7):
            nc.tensor.matmul(ps_warm, lhsT=ident16, rhs=warm_sb, start=True,
                             stop=True)

        # ---- persistent activations ----
        qk_sb = pers.tile([P, S], f16)    # rows 0:64 Q^T, rows 64:128 K^T
        kt2_sb = pers.tile([64, S], f16)  # K^T relocated to partitions 0-63
        # chunk 0 swapped-projection (rows 0:64 K^T, 64:128 Q^T): avoids any
        # relocation-DMA dependency on the startup critical path
        qk2_sb = pers.tile([P, QC], f16)
        if ROWPACK_SCORES:
            qt2_sb = pers.tile([P, S], f16)  # rows 64:128 = Q^T duplicate
        v_sb = pers.tile([P, NT, VW], f16)  # V natural tiles + ones column
        nc.vector.memset(v_sb[:, :, 64:65], 1.0)

        xtc_tiles = {}
        po_tiles = {}

        def emit_xtc_dma(c):
            if c >= NQ:
                return
            t = xt_pool.tile([P, ND, QC], f16, name="xtc", tag="xtc")
            nc.sync.dma_start(
                out=t,
                in_=xt_h[:, c * QC:(c + 1) * QC].rearrange(
                    "(dc p) s -> p dc s", p=P
                ),
            )
            xtc_tiles[c] = t

        def proj_gen(c):
            """Projection of chunk c as a generator of small PE bursts so it
            can be spread across the ACT-bound attention pair loop."""
            cs = slice(c * QC, (c + 1) * QC)
            xtc = xtc_tiles.pop(c)
            # Q,K projection (packed, M=128)
            ps_qk = ps_sm.tile([P, QC], f32, name="ps_qk", tag="sm")
            for dc in range(ND):
                nc.tensor.matmul(
                    ps_qk, lhsT=wqk_sb[:, dc, :], rhs=xtc[:, dc, :],
                    start=(dc == 0), stop=(dc == ND - 1),
                )
                if dc % 2 == 1:
                    yield
            nc.vector.tensor_copy(qk_sb[:, cs], ps_qk)
            # relocations (partition shifts need a DMA, not a DVE op);
            # gpsimd SWDGE keeps them off the sync ring used by x loads
            nc.gpsimd.dma_start(out=kt2_sb[:, cs], in_=qk_sb[64:128, cs])
            if ROWPACK_SCORES and c > 0:
                nc.gpsimd.dma_start(out=qt2_sb[64:128, cs], in_=qk_sb[0:64, cs])
            yield
            if c == 0:
                # swapped projection: chunk 0 scores read qk2_sb directly
                ps_q2 = ps_sm.tile([P, QC], f32, name="ps_q2", tag="sm")
                for dc in range(ND):
                    nc.tensor.matmul(
                        ps_q2, lhsT=wkq_sb[:, dc, :], rhs=xtc[:, dc, :],
                        start=(dc == 0), stop=(dc == ND - 1),
                    )
                    if dc % 4 == 3:
                        yield
                nc.vector.tensor_copy(qk2_sb, ps_q2)
                yield
            # V projection: two col-tiled halves run concurrently
            ps_va = ps_sm.tile([P, QC // 2], f32, name="ps_va", tag="sm")
            ps_vb = ps_sm.tile([P, QC // 2], f32, name="ps_vb", tag="sm")
            for dc in range(ND):
                st, sp = (dc == 0), (dc == ND - 1)
                nc.tensor.matmul(
                    ps_va[0:64, :], lhsT=wv_sb[:, dc, :],
                    rhs=xtc[:, dc, 0:QC // 2], start=st, stop=sp,
                )
                nc.tensor.matmul(
                    ps_vb[64:128, :], lhsT=wv_sb[:, dc, :],
                    rhs=xtc[:, dc, QC // 2:], start=st, stop=sp,
                    tile_position=(0, 64),
                )
                if dc % 4 == 3:
                    yield
            vt_sb = stage.tile([P, QC // 2], f16, name="vt_sb", tag="vt")
            nc.vector.tensor_copy(vt_sb[0:64, :], ps_va[0:64, :])
            nc.vector.tensor_copy(vt_sb[64:128, :], ps_vb[64:128, :])
            yield
            # transpose [128,128] once per half: rows 0:64 of the result
            # are t-tile 4c+k, rows 64:128 are t-tile 4c+2+k
            for k in range(2):
                ps_tv = ps_sm.tile([P, P], f16, name="ps_tv", tag="sm")
                nc.tensor.transpose(ps_tv, vt_sb[:, k * P:(k + 1) * P], ident16)
                nc.vector.tensor_copy(v_sb[:, 4 * c + k, 0:64], ps_tv[:, 0:64])
                nc.vector.tensor_copy(
                    v_sb[:, 4 * c + 2 + k, 0:64], ps_tv[:, 64:128]
                )
                yield
            # free an xt slot -> prefetch a later chunk
            emit_xtc_dma(c + 2)

        def emit_pair(c, jp, po):
            cs = slice(c * QC, (c + 1) * QC)
            njs = 4 * (c + 1)
            j0, j1 = 2 * jp, 2 * jp + 1
            jj = j0 - 4 * c
            trimmed = jj == 2  # second diagonal pair: >62% masked
            # pair 0 avoids the row-packed path so a fresh chunk's first
            # scores don't wait on the qt2 relocation DMA
            packed = ROWPACK_SCORES and jp > 0
            # chunk 0 reads its swapped projection instead of relocations
            def ktlo(j):
                if c == 0:
                    return qk2_sb[0:64, j * P:(j + 1) * P]
                return kt2_sb[:, j * P:(j + 1) * P]

            def qthi(lo):
                if c == 0:
                    return qk2_sb[64:128, lo:QC]
                return qt2_sb[64:128, c * QC + lo:(c + 1) * QC]

            ps_s = ps_pair.tile([P, 2 * QC], f32, name="ps_s", tag="pair")
            pt = pt_pool.tile([P, 2 * QC], f16, name="pt", tag="pt")
            if trimmed:
                # jj=2 half: only q in [256,512) is live; jj=3 half: only q in
                # [384,512), remapped to columns [512,640) so one ACT covers a
                # contiguous [256,640) region.
                nc.tensor.matmul(
                    ps_s[:, QC // 2:QC],
                    lhsT=ktlo(j0),
                    rhs=qk_sb[0:64, c * QC + QC // 2:(c + 1) * QC],
                    start=True, stop=True,
                )
                nc.tensor.matmul(
                    ps_s[:, QC:QC + QC // 4],
                    lhsT=qk_sb[64:128, j1 * P:(j1 + 1) * P],
                    rhs=qthi(3 * QC // 4),
                    start=True, stop=True, tile_position=(64, 0),
                )
                nc.vector.memset(pt[:, 0:QC // 2], 0.0)
                nc.scalar.activation(
                    pt[:, QC // 2:QC + QC // 4], ps_s[:, QC // 2:QC + QC // 4],
                    Exp, scale=SCALE,
                )
                nc.vector.tensor_tensor(
                    pt[:, QC // 2:QC], pt[:, QC // 2:QC],
                    dm_sb[:, 1, QC // 2:QC], op=mult,
                )
                nc.vector.tensor_tensor(
                    pt[:, QC:QC + QC // 4], pt[:, QC:QC + QC // 4],
                    dm_sb[:, 1, 2 * QC - QC // 4:], op=mult,
                )
                nc.tensor.matmul(
                    po[:, 3 * QC // 4:], lhsT=v_sb[:, j1, 0:65],
                    rhs=pt[:, QC:QC + QC // 4], start=False, stop=False,
                )
                nc.tensor.matmul(
                    po, lhsT=v_sb[:, j0, 0:65], rhs=pt[:, 0:QC],
                    start=False, stop=(j1 == njs - 1),
                )
                return
            nc.tensor.matmul(
                ps_s[:, 0:QC],
                lhsT=ktlo(j0), rhs=qk_sb[0:64, cs],
                start=True, stop=True,
            )
            if packed:
                nc.tensor.matmul(
                    ps_s[:, QC:],
                    lhsT=qk_sb[64:128, j1 * P:(j1 + 1) * P],
                    rhs=qthi(0),
                    start=True, stop=True, tile_position=(64, 0),
                )
            else:
                nc.tensor.matmul(
                    ps_s[:, QC:],
                    lhsT=ktlo(j1), rhs=qk_sb[0:64, cs],
                    start=True, stop=True,
                )
            nc.scalar.activation(pt, ps_s, Exp, scale=SCALE)
            if jj == 0:  # first diagonal pair
                nc.vector.tensor_tensor(pt, pt, dm_sb[:, 0, :], op=mult)
            nc.tensor.matmul(
                po, lhsT=v_sb[:, j0, 0:65], rhs=pt[:, 0:QC],
                start=(j0 == 0), stop=False,
            )
            nc.tensor.matmul(
                po, lhsT=v_sb[:, j1, 0:65], rhs=pt[:, QC:],
                start=False, stop=(j1 == njs - 1),
            )

        def epi_gen(c):
            cs = slice(c * QC, (c + 1) * QC)
            po = po_tiles.pop(c)
            u_sb = stage.tile([65, QC], f16, name="u_sb", tag="u")
            nc.vector.tensor_copy(u_sb, po)
            yield
            osb = stage.tile([P, 4, DK], f32, name="osb", tag="osb")
            # batched: 4 transposes into one PSUM tile, one vectorized
            # reciprocal, then the per-subtile normalizing multiplies
            ps_t4 = ps_sm.tile([P, 4, 66], f16, name="ps_t4", tag="sm")
            for k in range(4):
                nc.tensor.transpose(
                    ps_t4[:, k, 0:65], u_sb[:, k * P:(k + 1) * P],
                    ident16[0:65, 0:65],
                )
            yield
            rinv4 = stage.tile([P, 4, 1], f32, name="rinv4", tag="rinv")
            nc.vector.reciprocal(rinv4, ps_t4[:, :, 64:65])
            yield
            for k in range(4):
                nc.vector.tensor_scalar_mul(
                    osb[:, k, :], ps_t4[:, k, 0:64], rinv4[:, k, :]
                )
                if k == 1:
                    yield
            nc.gpsimd.dma_start(
                out=o_h[cs, :].rearrange("(k p) d -> p k d", p=P), in_=osb
            )

        # Software pipeline: a global queue of deferrable PE work
        # (projections of later chunks, epilogues of finished chunks) is
        # drained in small bursts between attention pairs, so the PE fills
        # its exp-wait slack and never idles across chunk boundaries.
        proj_pending = {}   # chunk -> generator (deadline: chunk start)
        epi_pending = []    # generators (no deadline)

        def pull_one(max_chunk=None):
            while proj_pending:
                c0 = min(proj_pending)
                if max_chunk is not None and c0 > max_chunk:
                    break
                try:
                    next(proj_pending[c0])
                    return
                except StopIteration:
                    del proj_pending[c0]
            while epi_pending:
                try:
                    next(epi_pending[0])
                    return
                except StopIteration:
                    epi_pending.pop(0)

        def ensure_proj(c):
            g = proj_pending.pop(c, None)
            if g is not None:
                for _ in g:
                    pass

        xtc_tiles[0] = xtc0
        emit_xtc_dma(1)
        for _ in proj_gen(0):
            pass
        for c in range(1, NQ):
            proj_pending[c] = proj_gen(c)
        for c in range(NQ):
            ensure_proj(c)
            po = ps_op.tile([65, QC], f32, name="po", tag="po")
            po_tiles[c] = po
            for jp in range(2 * (c + 1)):
                emit_pair(c, jp, po)
                # front-load the next projection into the first pairs so it
                # (and its relocation DMAs) completes well before the boundary
                pulls = 3 if jp < 4 else 1
                for _ in range(pulls):
                    pull_one(max_chunk=c + 1)
            epi_pending.append(epi_gen(c))
        for _, g in sorted(proj_pending.items()):
            for _ in g:
                pass
        proj_pending.clear()
        for g in epi_pending:
            for _ in g:
                pass
    nc.finalize()
    return nc


def _host_inputs(x, wQ, wK, wV):
    x = np.asarray(x, dtype=np.float32)
    wqk = np.ascontiguousarray(
        np.concatenate([np.asarray(wQ).T, np.asarray(wK).T], axis=1)
    ).astype(np.float16)
    wkq = np.ascontiguousarray(
        np.concatenate([np.asarray(wK).T, np.asarray(wQ).T], axis=1)
    ).astype(np.float16)
    wv = np.ascontiguousarray(np.asarray(wV).T).astype(np.float16)
    chain = np.zeros((P, P), np.float32)
    in_maps = []
    for b in range(B):
        xt = np.ascontiguousarray(x[b].T).astype(np.float16)
        in_maps.append(
            {"xt": xt, "wqk": wqk, "wkq": wkq, "wv": wv, "chain": chain}
        )
    return in_maps


def kernel(x, wQ, wK, wV):
    from concourse.bass_utils import run_bass_kernel_spmd

    if "nc" not in _CACHED:
        _CACHED["nc"] = _build_nc()
    nc = _CACHED["nc"]
    in_maps = _host_inputs(x, wQ, wK, wV)
    res = run_bass_kernel_spmd(nc, in_maps, core_ids=list(range(B)))
    out = np.stack([res.results[i]["o"] for i in range(B)]).astype(np.float32)
    return out
